# revision 13
# baseline (speedup 1.0000x reference)
"""LocationSensitiveSoftAttention on 8 Trainium2 NeuronCores (Bass/Tile).

Contract: kernel(**inputs) takes the FULL unsharded inputs (numpy arrays, keys
as in setup_inputs()) and returns the FULL output [64, 1, 256] fp32.

Strategy: data-parallel over batch B=64 -> 8 batches per core; weights
replicated. Math restructure (exact up to fp rounding):
  pre[t,:] = mem[t,:] @ (Wm@We) + sum_k spad[t+k] * CW[k,:] + r
    where CW = conv_w^T @ (Wl@We)  (conv folded into loc projection)
          r  = q1@Wq@We + (bq+bm+bl)@We + be + conv_b@(Wl@We)
  h = tanh(pre); energy = h @ v_a; s = sigmoid(energy)
  w = state + s/sum(s)
  context = (w @ mem) @ Wm + (sum(state) + 1) * bm

Precision: the attention path (pre GEMM) runs in fp8-e4m3 with DoubleRow
(2x contraction per pass); its error is damped ~1000x in the output because
sum(state)~1024 vs sum(alignment)=1. The context matvec reads an e3m4 copy
of mem (4-bit mantissa) against bf16 weights; measured end-to-end rel err
~1.2e-2 on the fixed inputs. Scales (mem*2, WmWe*32) keep fp8 out of the
subnormal range and are folded back via the tanh activation scale=1/64.

The context matvec for 4 batches runs concurrently on the PE via
tile_position column-tiling (output partitions 0/32/64/96).
"""

import sys

for _p in ("/root/.axon_site", "/root/.axon_site/_ro/trn_rl_repo",
           "/root/.axon_site/_ro/pypackages", "/opt/trn_rl_repo"):
    if _p not in sys.path:
        sys.path.append(_p)

import numpy as np
import ml_dtypes

B, TQ, T = 64, 2, 2048
HID, ENC, U, FILT, K = 1024, 512, 256, 32, 31
N_CORES = 8
PB = B // N_CORES  # batches per core
PAD = K // 2  # 15
NT = T // 128  # 16 t-tiles
NBLK = T // 512  # 4 t-blocks

BF16 = ml_dtypes.bfloat16
E4 = ml_dtypes.float8_e4m3
E3 = ml_dtypes.float8_e3m4

_BUILT = {}
TRACE = False
LAST_RESULTS = None


def _build_nc(repeat=1):
    import concourse.bacc as bacc
    import concourse.mybir as mybir
    import concourse.tile as tile
    import concourse.bass as bass

    f32 = mybir.dt.float32
    bf16 = mybir.dt.bfloat16
    e4 = mybir.dt.float8e4
    e3 = mybir.dt.float8e3
    AF = mybir.ActivationFunctionType
    ALU = mybir.AluOpType
    AX = mybir.AxisListType
    DR = mybir.MatmulPerfMode.DoubleRow

    nc = bacc.Bacc("TRN2", target_bir_lowering=False, debug=False,
                   num_devices=N_CORES)

    # ---- DRAM I/O ----
    memt8_d = nc.dram_tensor("memt8", [PB, 128, 4, T], e4, kind="ExternalInput")
    nat83_d = nc.dram_tensor("nat83", [PB, NT, 128, ENC], e3, kind="ExternalInput")
    spadb_d = nc.dram_tensor("spadb", [PB, T + 2 * PAD], bf16, kind="ExternalInput")
    spad_d = nc.dram_tensor("spad", [PB, T + 2 * PAD], f32, kind="ExternalInput")
    statet_d = nc.dram_tensor("statet", [128, PB, NT], f32, kind="ExternalInput")
    q1_d = nc.dram_tensor("q1", [PB, HID], f32, kind="ExternalInput")
    wmwe8_d = nc.dram_tensor("wmwe8", [ENC, U], e4, kind="ExternalInput")
    cw_d = nc.dram_tensor("cw", [K, U], bf16, kind="ExternalInput")
    c0_d = nc.dram_tensor("c0", [1, U], bf16, kind="ExternalInput")
    wq_d = nc.dram_tensor("wq", [HID, U], bf16, kind="ExternalInput")
    we_d = nc.dram_tensor("we", [U, U], bf16, kind="ExternalInput")
    wm_d = nc.dram_tensor("wm", [ENC, U], f32, kind="ExternalInput")
    bm_d = nc.dram_tensor("bm", [1, U], f32, kind="ExternalInput")
    vaT_d = nc.dram_tensor("vaT", [128, 2], bf16, kind="ExternalInput")
    idf_d = nc.dram_tensor("idf", [128, 128], f32, kind="ExternalInput")
    idb_d = nc.dram_tensor("idb", [NT, NT], bf16, kind="ExternalInput")
    out_d = nc.dram_tensor("out", [PB, U], f32, kind="ExternalOutput")

    with tile.TileContext(nc) as tc:
        with (
            tc.tile_pool(name="consts", bufs=1) as consts,
            tc.tile_pool(name="mt8", bufs=2) as mt8p,
            tc.tile_pool(name="nat", bufs=6) as natp,
            tc.tile_pool(name="shp", bufs=3) as shp,
            tc.tile_pool(name="stp", bufs=3) as stp,
            tc.tile_pool(name="hb", bufs=2) as hbp,
            tc.tile_pool(name="rows", bufs=2) as rowp,
            tc.tile_pool(name="wtp", bufs=6) as wtp,
            tc.tile_pool(name="psA", bufs=2, space="PSUM") as psA,
            tc.tile_pool(name="psB", bufs=2, space="PSUM") as psB,
            tc.tile_pool(name="psC", bufs=2, space="PSUM") as psC,
            tc.tile_pool(name="psD", bufs=2, space="PSUM") as psD,
        ):
          def _body():
              # ---- load constants ----
              wmwe8_sb = consts.tile([128, 4, U], e4, tag="wmwe8")
              nc.scalar.dma_start(out=wmwe8_sb[:], in_=bass.AP(
                  tensor=wmwe8_d, offset=0,
                  ap=[[U, 128], [128 * U, 4], [1, U]]))
              wq_sb = []
              for j in range(8):
                  t_ = consts.tile([128, U], bf16, tag=f"wq{j}")
                  nc.scalar.dma_start(out=t_[:], in_=wq_d.ap()[j * 128:(j + 1) * 128, :])
                  wq_sb.append(t_)
              we_sb = []
              for j in range(2):
                  t_ = consts.tile([128, U], bf16, tag=f"we{j}")
                  nc.scalar.dma_start(out=t_[:], in_=we_d.ap()[j * 128:(j + 1) * 128, :])
                  we_sb.append(t_)
              wm_sb = []
              for ec in range(4):
                  t_ = consts.tile([128, U], f32, tag=f"wm{ec}")
                  nc.scalar.dma_start(out=t_[:], in_=wm_d.ap()[ec * 128:(ec + 1) * 128, :])
                  wm_sb.append(t_)
              cw_sb = consts.tile([K, U], bf16, tag="cw")
              nc.scalar.dma_start(out=cw_sb[:], in_=cw_d.ap())
              c0_sb = consts.tile([1, U], bf16, tag="c0")
              nc.scalar.dma_start(out=c0_sb[:], in_=c0_d.ap())
              bm_sb = consts.tile([1, U], f32, tag="bm")
              nc.scalar.dma_start(out=bm_sb[:], in_=bm_d.ap())
              vaT_sb = consts.tile([128, 2], bf16, tag="vaT")
              nc.scalar.dma_start(out=vaT_sb[:], in_=vaT_d.ap())
              idf_sb = consts.tile([128, 128], f32, tag="idf")
              nc.scalar.dma_start(out=idf_sb[:], in_=idf_d.ap())
              idb_sb = consts.tile([NT, NT], bf16, tag="idb")
              nc.scalar.dma_start(out=idb_sb[:], in_=idb_d.ap())
              state_sb = consts.tile([PB, T + 2 * PAD], f32, tag="state")
              nc.scalar.dma_start(out=state_sb[:], in_=spad_d.ap())
              ones8 = consts.tile([1, 8], bf16, tag="ones8")
              nc.vector.memset(ones8[:], 1.0)
              ones_col = consts.tile([128, 1], f32, tag="onesc")
              nc.vector.memset(ones_col[:], 1.0)
              ones_row = consts.tile([1, 128], f32, tag="onesr")
              nc.vector.memset(ones_row[:], 1.0)
              call_sb = consts.tile([PB, ENC], f32, tag="call")

              # ---- sum(state) + 1 row [1, 8] ----
              stsum = consts.tile([PB, 1], f32, tag="stsum")
              nc.vector.tensor_reduce(stsum[:], state_sb[:, PAD:PAD + T],
                                      axis=AX.X, op=ALU.add)
              ps_sig = psC.tile([1, 8], f32, tag="misc")
              nc.tensor.matmul(ps_sig[:], stsum[:], idf_sb[0:PB, 0:PB],
                               is_transpose=True)
              sig_row = consts.tile([1, 8], f32, tag="sigrow")
              nc.vector.tensor_scalar_add(sig_row[:], ps_sig[:], 1.0)

              # ---- pq -> r rows, transposed to rT2 [128, 2, 16] ----
              q1_sb = consts.tile([PB, HID], f32, tag="q1")
              nc.scalar.dma_start(out=q1_sb[:], in_=q1_d.ap())
              q1_bf = consts.tile([16, HID], bf16, tag="q1bf")
              nc.vector.memset(q1_bf[:], 0.0)
              nc.vector.tensor_copy(q1_bf[0:PB, :], q1_sb[:])
              q1T2 = consts.tile([128, 8, 16], bf16, tag="q1T2")
              nc.sync.dma_start(out=q1T2[:], in_=q1_bf[:], transpose=True)
              pq_ps = psC.tile([PB, U], f32, tag="misc")
              for j in range(8):
                  nc.tensor.matmul(pq_ps[:], q1T2[:, j, 0:PB], wq_sb[j][:],
                                   start=(j == 0), stop=(j == 7))
              pq_bf = consts.tile([16, U], bf16, tag="pqbf")
              nc.vector.memset(pq_bf[:], 0.0)
              nc.scalar.activation(pq_bf[0:PB, :], pq_ps[:], AF.Copy)
              pqT2 = consts.tile([128, 2, 16], bf16, tag="pqT2")
              nc.sync.dma_start(out=pqT2[:], in_=pq_bf[:], transpose=True)
              r_ps = psC.tile([PB, U], f32, tag="misc")
              nc.tensor.matmul(r_ps[:], pqT2[:, 0, 0:PB], we_sb[0][:],
                               start=True, stop=False)
              nc.tensor.matmul(r_ps[:], pqT2[:, 1, 0:PB], we_sb[1][:],
                               start=False, stop=False)
              nc.tensor.matmul(r_ps[:], ones8[:], c0_sb[:], start=False, stop=True)
              r_bf = consts.tile([16, U], bf16, tag="rbf")
              nc.vector.memset(r_bf[:], 0.0)
              nc.scalar.activation(r_bf[0:PB, :], r_ps[:], AF.Copy)
              rT2 = consts.tile([128, 2, 16], bf16, tag="rT2")
              nc.sync.dma_start(out=rT2[:], in_=r_bf[:], transpose=True)

              statet_sb = consts.tile([128, PB, NT], f32, tag="statet")
              nc.scalar.dma_start(out=statet_sb[:], in_=bass.AP(
                  tensor=statet_d, offset=0,
                  ap=[[PB * NT, 128], [NT, PB], [1, NT]]))

              # ---- per-batch pipeline ----
              def load_b(b):
                  st = {}
                  mt8 = mt8p.tile([128, 4, T], e4, tag="mt8", name=f"mt8_{b}")
                  nc.sync.dma_start(out=mt8[:], in_=memt8_d.ap()[b])
                  nat = natp.tile([128, NT, ENC], e3, tag="nat", name=f"nat{b}")
                  nc.sync.dma_start(out=nat[:], in_=bass.AP(
                      tensor=nat83_d, offset=b * NT * 128 * ENC,
                      ap=[[ENC, 128], [128 * ENC, NT], [1, ENC]]))
                  sh = shp.tile([K, T], bf16, tag="sh", name=f"sh{b}")
                  nc.sync.dma_start(out=sh[:], in_=bass.AP(
                      tensor=spadb_d, offset=b * (T + 2 * PAD),
                      ap=[[1, K], [1, T]]))
                  st.update(mt8=mt8, nat=nat, sh=sh)
                  return st

              def attn_b(b, st):
                  """Attention matmuls + tanh + energy; no slow round-trips on
                  the PE queue. Leaves en2d (energy rows, bf16) in st."""
                  mt8, sh = st["mt8"], st["sh"]
                  h_tiles = [[None] * NBLK for _ in range(2)]
                  for vch in range(2):
                      uo = vch * 128
                      for tb in range(NBLK):
                          to = tb * 512
                          ps = psA.tile([128, 512], f32, tag="pre")
                          nc.tensor.matmul(ps[:], wmwe8_sb[:, 0:2, uo:uo + 128],
                                           mt8[:, 0:2, to:to + 512],
                                           start=True, stop=False, perf_mode=DR)
                          nc.tensor.matmul(ps[:], wmwe8_sb[:, 2:4, uo:uo + 128],
                                           mt8[:, 2:4, to:to + 512],
                                           start=False, stop=False, perf_mode=DR)
                          nc.tensor.matmul(ps[:], cw_sb[:, uo:uo + 128],
                                           sh[:, to:to + 512],
                                           start=False, stop=True)
                          hb = hbp.tile([128, 512], bf16, tag=f"h{vch}{tb}")
                          nc.scalar.activation(hb[:], ps[:], AF.Tanh,
                                               bias=rT2[:, vch, b:b + 1],
                                               scale=1.0 / 64)
                          h_tiles[vch][tb] = hb
                  # 4 col-tiled energy matvecs run concurrently on the PE
                  en_ps = psB.tile([128, 512], f32, tag="en")
                  for vch in range(2):
                      for tb in range(NBLK):
                          nc.tensor.matmul(
                              en_ps[32 * tb:32 * tb + 1, :],
                              vaT_sb[:, vch:vch + 1], h_tiles[vch][tb][:],
                              start=(vch == 0), stop=(vch == 1),
                              tile_position=(0, 32 * tb),
                              skip_group_check=True)
                  en_rows = rowp.tile([128, 512], bf16, tag="enrows")
                  en2d = rowp.tile([NT, 128], bf16, tag="en2d",
                                   name=f"en2d{b}", bufs=3)
                  for tb in range(NBLK):
                      nc.vector.tensor_copy(en_rows[32 * tb:32 * tb + 1, :],
                                            en_ps[32 * tb:32 * tb + 1, :])
                      nc.gpsimd.dma_start(out=en2d[4 * tb:4 * tb + 4, :],
                                            in_=en_rows[32 * tb:32 * tb + 1, :])
                  st["en2d"] = en2d

              def wfin_b(b, st):
                  """Finish w = state + s/sum(s): emitted one batch late so the
                  PE transpose/sum never stall the PE queue."""
                  enT_ps = psC.tile([128, NT], bf16, tag="misc")
                  nc.tensor.matmul(enT_ps[:], st["en2d"][:], idb_sb[:],
                                   is_transpose=True)
                  sT = rowp.tile([128, NT], bf16, tag="sT", name=f"sT{b}",
                                 bufs=3)
                  ssumP = rowp.tile([128, 1], f32, tag="ssumP")
                  nc.scalar.activation(sT[:], enT_ps[:], AF.Sigmoid,
                                       accum_out=ssumP[:])
                  tot_ps = psC.tile([1, 1], f32, tag="misc")
                  nc.tensor.matmul(tot_ps[:], ssumP[:], ones_col[:],
                                   start=True, stop=True)
                  tot = rowp.tile([1, 1], f32, tag="tot")
                  nc.vector.tensor_copy(tot[:], tot_ps[:])
                  rec = rowp.tile([1, 1], f32, tag="rec", name=f"rec{b}",
                                  bufs=3)
                  nc.vector.reciprocal(rec[:], tot[:])
                  recB_ps = psC.tile([128, 1], f32, tag="misc")
                  nc.tensor.matmul(recB_ps[:], ones_row[:], rec[:],
                                   start=True, stop=True)
                  recB = rowp.tile([128, 1], f32, tag="recB", name=f"recB{b}",
                                   bufs=3)
                  nc.vector.tensor_copy(recB[:], recB_ps[:])
                  wT = wtp.tile([128, NT], bf16, tag="wT", name=f"wT{b}")
                  nc.vector.scalar_tensor_tensor(
                      wT[:], in0=sT[:], scalar=recB[:],
                      in1=statet_sb[:, b, :],
                      op0=ALU.mult, op1=ALU.add)
                  st["wT"] = wT

              def context_group(g, sts):
                  ctx_ps = psD.tile([128, 512], f32, tag="ctx")
                  for ch in range(NT):
                      for j in range(4):
                          stj = sts[4 * g + j]
                          nc.tensor.matmul(
                              ctx_ps[32 * j:32 * j + 1, :],
                              stj["wT"][:, ch:ch + 1],
                              stj["nat"][:, ch, :],
                              start=(ch == 0), stop=(ch == NT - 1),
                              tile_position=(0, 32 * j),
                              skip_group_check=True)
                  cv_sb = rowp.tile([128, 512], f32, tag="cvsb")
                  for j in range(4):
                      nc.vector.tensor_copy(cv_sb[32 * j:32 * j + 1, :],
                                            ctx_ps[32 * j:32 * j + 1, :])
                  for j in range(4):
                      nc.gpsimd.dma_start(
                          out=call_sb[4 * g + j:4 * g + j + 1, :],
                          in_=cv_sb[32 * j:32 * j + 1, :])

              sts = [load_b(0)]
              pend = None
              for b in range(PB):
                  if b + 1 < PB:
                      sts.append(load_b(b + 1))
                  attn_b(b, sts[b])
                  if pend is not None:
                      wfin_b(pend, sts[pend])
                  pend = b
                  if b == 5:
                      context_group(0, sts)
              wfin_b(7, sts[7])
              context_group(1, sts)

              # ---- final: context = Call @ Wm + sig_row^T * bm ----
              callT = []
              for ch in range(4):
                  pst = psC.tile([128, PB], f32, tag="misc")
                  nc.tensor.matmul(pst[:], call_sb[:, ch * 128:(ch + 1) * 128],
                                   idf_sb[0:PB, 0:PB], is_transpose=True)
                  t_ = consts.tile([128, PB], f32, tag=f"callT{ch}")
                  nc.vector.tensor_copy(t_[:], pst[:])
                  callT.append(t_)
              ctx_ps = psC.tile([PB, U], f32, tag="misc")
              for ch in range(4):
                  nc.tensor.matmul(ctx_ps[:], callT[ch][:], wm_sb[ch][:],
                                   start=(ch == 0), stop=False)
              nc.tensor.matmul(ctx_ps[:], sig_row[:], bm_sb[:],
                               start=False, stop=True)
              ctx_sb = consts.tile([PB, U], f32, tag="ctx")
              nc.vector.tensor_copy(ctx_sb[:], ctx_ps[:])
              nc.sync.dma_start(out=out_d.ap(), in_=ctx_sb[:])

          for _rep in range(repeat):
              _body()
    nc.compile()
    return nc


def _host_prep(inputs):
    """Fold weights on host (weight-only transforms) and shard per core."""
    f32 = np.float32
    Wq = np.asarray(inputs["Wq"], f32)
    bq = np.asarray(inputs["bq"], f32)
    Wm = np.asarray(inputs["Wm"], f32)
    bm = np.asarray(inputs["bm"], f32)
    Wl = np.asarray(inputs["Wl"], f32)
    bl = np.asarray(inputs["bl"], f32)
    conv_w = np.asarray(inputs["conv_w"], f32)
    conv_b = np.asarray(inputs["conv_b"], f32)
    We = np.asarray(inputs["We"], f32)
    be = np.asarray(inputs["be"], f32)
    v_a = np.asarray(inputs["v_a"], f32)

    WmWe = (Wm @ We).astype(f32)
    WlWe = (Wl @ We).astype(f32)
    CW64 = (conv_w[:, 0, :].T @ WlWe * 64.0).astype(f32)
    c0 = ((bq + bm + bl) @ We + be + conv_b @ WlWe).astype(f32)

    query = np.asarray(inputs["query"], f32)
    state = np.asarray(inputs["state"], f32)
    memory = np.ascontiguousarray(np.asarray(inputs["memory"], f32))

    spad = np.zeros((B, T + 2 * PAD), f32)
    spad[:, PAD:PAD + T] = state
    q1 = np.ascontiguousarray(query[:, 1, :])

    ident = np.eye(128, dtype=f32)
    shared = {
        "wmwe8": (WmWe * 32.0).astype(E4),
        "cw": CW64.astype(BF16),
        "c0": c0.reshape(1, U).astype(BF16),
        "wq": Wq.astype(BF16),
        "we": We.astype(BF16),
        "wm": Wm.astype(f32),
        "bm": bm.reshape(1, U).astype(f32),
        "vaT": np.ascontiguousarray(v_a.reshape(2, 128).T).astype(BF16),
        "idf": ident,
        "idb": np.eye(NT, dtype=np.float32).astype(BF16),
    }
    in_maps = []
    for c in range(N_CORES):
        sl = slice(c * PB, (c + 1) * PB)
        m = dict(shared)
        mb = memory[sl]
        m["memt8"] = np.ascontiguousarray(
            (mb * 2.0).reshape(PB, T, 4, 128).transpose(0, 3, 2, 1)).astype(E4)
        m["nat83"] = np.ascontiguousarray(
            mb.reshape(PB, NT, 128, ENC)).astype(E3)
        m["spad"] = np.ascontiguousarray(spad[sl])
        m["spadb"] = np.ascontiguousarray(spad[sl]).astype(BF16)
        m["statet"] = np.ascontiguousarray(
            state[sl].reshape(PB, NT, 128).transpose(2, 0, 1))
        m["q1"] = np.ascontiguousarray(q1[sl])
        in_maps.append(m)
    return in_maps


def kernel(**inputs) -> np.ndarray:
    global LAST_RESULTS
    from concourse import bass_utils

    if "nc" not in _BUILT:
        _BUILT["nc"] = _build_nc()
    nc = _BUILT["nc"]

    in_maps = _host_prep(inputs)
    res = bass_utils.run_bass_kernel_spmd(
        nc, in_maps, core_ids=list(range(N_CORES)), trace=TRACE)
    LAST_RESULTS = res
    out = np.concatenate([res.results[c]["out"] for c in range(N_CORES)], axis=0)
    return out.reshape(B, 1, U).astype(np.float32)


# revision 14
# speedup vs baseline: 1.0410x; 1.0410x over previous
"""LocationSensitiveSoftAttention on 8 Trainium2 NeuronCores (Bass/Tile).

Contract: kernel(**inputs) takes the FULL unsharded inputs (numpy arrays, keys
as in setup_inputs()) and returns the FULL output [64, 1, 256] fp32.

Strategy: data-parallel over batch B=64 -> 8 batches per core; weights
replicated. Math restructure (exact up to fp rounding):
  pre[t,:] = mem[t,:] @ (Wm@We) + sum_k spad[t+k] * CW[k,:] + r
    where CW = conv_w^T @ (Wl@We)  (conv folded into loc projection)
          r  = q1@Wq@We + (bq+bm+bl)@We + be + conv_b@(Wl@We)
  h = tanh(pre); energy = h @ v_a; s = sigmoid(energy)
  w = state + s/sum(s)
  context = (w @ mem) @ Wm + (sum(state) + 1) * bm

Precision: the attention path (pre GEMM) runs in fp8-e4m3 with DoubleRow
(2x contraction per pass); its error is damped ~1000x in the output because
sum(state)~1024 vs sum(alignment)=1. The context matvec reads an e3m4 copy
of mem (4-bit mantissa) against bf16 weights; measured end-to-end rel err
~1.2e-2 on the fixed inputs. Scales (mem*2, WmWe*32) keep fp8 out of the
subnormal range and are folded back via the tanh activation scale=1/64.

The context matvec for 4 batches runs concurrently on the PE via
tile_position column-tiling (output partitions 0/32/64/96).
"""

import sys

for _p in ("/root/.axon_site", "/root/.axon_site/_ro/trn_rl_repo",
           "/root/.axon_site/_ro/pypackages", "/opt/trn_rl_repo"):
    if _p not in sys.path:
        sys.path.append(_p)

import numpy as np
import ml_dtypes

B, TQ, T = 64, 2, 2048
HID, ENC, U, FILT, K = 1024, 512, 256, 32, 31
N_CORES = 8
PB = B // N_CORES  # batches per core
PAD = K // 2  # 15
NT = T // 128  # 16 t-tiles
NBLK = T // 512  # 4 t-blocks

BF16 = ml_dtypes.bfloat16
E4 = ml_dtypes.float8_e4m3
E3 = ml_dtypes.float8_e3m4

_BUILT = {}
TRACE = False
LAST_RESULTS = None


def _build_nc(repeat=1):
    import concourse.bacc as bacc
    import concourse.mybir as mybir
    import concourse.tile as tile
    import concourse.bass as bass

    f32 = mybir.dt.float32
    bf16 = mybir.dt.bfloat16
    e4 = mybir.dt.float8e4
    e3 = mybir.dt.float8e3
    AF = mybir.ActivationFunctionType
    ALU = mybir.AluOpType
    AX = mybir.AxisListType
    DR = mybir.MatmulPerfMode.DoubleRow

    nc = bacc.Bacc("TRN2", target_bir_lowering=False, debug=False,
                   num_devices=N_CORES)

    # ---- DRAM I/O ----
    memt8_d = nc.dram_tensor("memt8", [PB, 128, 4, T], e4, kind="ExternalInput")
    nat83_d = nc.dram_tensor("nat83", [PB, NT, 128, ENC], e3, kind="ExternalInput")
    spadb_d = nc.dram_tensor("spadb", [PB, T + 2 * PAD], bf16, kind="ExternalInput")
    spad_d = nc.dram_tensor("spad", [PB, T + 2 * PAD], f32, kind="ExternalInput")
    statet_d = nc.dram_tensor("statet", [128, PB, NT], f32, kind="ExternalInput")
    q1_d = nc.dram_tensor("q1", [PB, HID], f32, kind="ExternalInput")
    wmwe8_d = nc.dram_tensor("wmwe8", [ENC, U], e4, kind="ExternalInput")
    cw_d = nc.dram_tensor("cw", [K, U], bf16, kind="ExternalInput")
    c0_d = nc.dram_tensor("c0", [1, U], bf16, kind="ExternalInput")
    wq_d = nc.dram_tensor("wq", [HID, U], bf16, kind="ExternalInput")
    we_d = nc.dram_tensor("we", [U, U], bf16, kind="ExternalInput")
    wm_d = nc.dram_tensor("wm", [ENC, U], f32, kind="ExternalInput")
    bm_d = nc.dram_tensor("bm", [1, U], f32, kind="ExternalInput")
    vaT_d = nc.dram_tensor("vaT", [128, 2], bf16, kind="ExternalInput")
    idf_d = nc.dram_tensor("idf", [128, 128], f32, kind="ExternalInput")
    idb_d = nc.dram_tensor("idb", [NT, NT], bf16, kind="ExternalInput")
    out_d = nc.dram_tensor("out", [PB, U], f32, kind="ExternalOutput")

    with tile.TileContext(nc) as tc:
        with (
            tc.tile_pool(name="consts", bufs=1) as consts,
            tc.tile_pool(name="mt8", bufs=2) as mt8p,
            tc.tile_pool(name="nat", bufs=6) as natp,
            tc.tile_pool(name="shp", bufs=3) as shp,
            tc.tile_pool(name="stp", bufs=3) as stp,
            tc.tile_pool(name="hb", bufs=2) as hbp,
            tc.tile_pool(name="rows", bufs=2) as rowp,
            tc.tile_pool(name="wtp", bufs=6) as wtp,
            tc.tile_pool(name="psA", bufs=2, space="PSUM") as psA,
            tc.tile_pool(name="psB", bufs=2, space="PSUM") as psB,
            tc.tile_pool(name="psC", bufs=2, space="PSUM") as psC,
            tc.tile_pool(name="psD", bufs=2, space="PSUM") as psD,
        ):
          def _body():
              # ---- load constants ----
              wmwe8_sb = consts.tile([128, 4, U], e4, tag="wmwe8")
              nc.scalar.dma_start(out=wmwe8_sb[:], in_=bass.AP(
                  tensor=wmwe8_d, offset=0,
                  ap=[[U, 128], [128 * U, 4], [1, U]]))
              wq_sb = []
              for j in range(8):
                  t_ = consts.tile([128, U], bf16, tag=f"wq{j}")
                  nc.scalar.dma_start(out=t_[:], in_=wq_d.ap()[j * 128:(j + 1) * 128, :])
                  wq_sb.append(t_)
              we_sb = []
              for j in range(2):
                  t_ = consts.tile([128, U], bf16, tag=f"we{j}")
                  nc.scalar.dma_start(out=t_[:], in_=we_d.ap()[j * 128:(j + 1) * 128, :])
                  we_sb.append(t_)
              wm_sb = []
              for ec in range(4):
                  t_ = consts.tile([128, U], f32, tag=f"wm{ec}")
                  nc.scalar.dma_start(out=t_[:], in_=wm_d.ap()[ec * 128:(ec + 1) * 128, :])
                  wm_sb.append(t_)
              cw_sb = consts.tile([K, U], bf16, tag="cw")
              nc.scalar.dma_start(out=cw_sb[:], in_=cw_d.ap())
              c0_sb = consts.tile([1, U], bf16, tag="c0")
              nc.scalar.dma_start(out=c0_sb[:], in_=c0_d.ap())
              bm_sb = consts.tile([1, U], f32, tag="bm")
              nc.scalar.dma_start(out=bm_sb[:], in_=bm_d.ap())
              vaT_sb = consts.tile([128, 2], bf16, tag="vaT")
              nc.scalar.dma_start(out=vaT_sb[:], in_=vaT_d.ap())
              idf_sb = consts.tile([128, 128], f32, tag="idf")
              nc.scalar.dma_start(out=idf_sb[:], in_=idf_d.ap())
              idb_sb = consts.tile([NT, NT], bf16, tag="idb")
              nc.scalar.dma_start(out=idb_sb[:], in_=idb_d.ap())
              state_sb = consts.tile([PB, T + 2 * PAD], f32, tag="state")
              nc.scalar.dma_start(out=state_sb[:], in_=spad_d.ap())
              ones8 = consts.tile([1, 8], bf16, tag="ones8")
              nc.vector.memset(ones8[:], 1.0)
              ones_col = consts.tile([128, 1], f32, tag="onesc")
              nc.vector.memset(ones_col[:], 1.0)
              ones_row = consts.tile([1, 128], f32, tag="onesr")
              nc.vector.memset(ones_row[:], 1.0)
              call_sb = consts.tile([PB, ENC], f32, tag="call")

              # ---- sum(state) + 1 row [1, 8] ----
              stsum = consts.tile([PB, 1], f32, tag="stsum")
              nc.vector.tensor_reduce(stsum[:], state_sb[:, PAD:PAD + T],
                                      axis=AX.X, op=ALU.add)
              ps_sig = psC.tile([1, 8], f32, tag="misc")
              nc.tensor.matmul(ps_sig[:], stsum[:], idf_sb[0:PB, 0:PB],
                               is_transpose=True)
              sig_row = consts.tile([1, 8], f32, tag="sigrow")
              nc.vector.tensor_scalar_add(sig_row[:], ps_sig[:], 1.0)

              # ---- pq -> r rows, transposed to rT2 [128, 2, 16] ----
              q1_sb = consts.tile([PB, HID], f32, tag="q1")
              nc.scalar.dma_start(out=q1_sb[:], in_=q1_d.ap())
              q1_bf = consts.tile([16, HID], bf16, tag="q1bf")
              nc.vector.memset(q1_bf[:], 0.0)
              nc.vector.tensor_copy(q1_bf[0:PB, :], q1_sb[:])
              q1T2 = consts.tile([128, 8, 16], bf16, tag="q1T2")
              nc.sync.dma_start(out=q1T2[:], in_=q1_bf[:], transpose=True)
              pq_ps = psC.tile([PB, U], f32, tag="misc")
              for j in range(8):
                  nc.tensor.matmul(pq_ps[:], q1T2[:, j, 0:PB], wq_sb[j][:],
                                   start=(j == 0), stop=(j == 7))
              pq_bf = consts.tile([16, U], bf16, tag="pqbf")
              nc.vector.memset(pq_bf[:], 0.0)
              nc.scalar.activation(pq_bf[0:PB, :], pq_ps[:], AF.Copy)
              pqT2 = consts.tile([128, 2, 16], bf16, tag="pqT2")
              nc.sync.dma_start(out=pqT2[:], in_=pq_bf[:], transpose=True)
              r_ps = psC.tile([PB, U], f32, tag="misc")
              nc.tensor.matmul(r_ps[:], pqT2[:, 0, 0:PB], we_sb[0][:],
                               start=True, stop=False)
              nc.tensor.matmul(r_ps[:], pqT2[:, 1, 0:PB], we_sb[1][:],
                               start=False, stop=False)
              nc.tensor.matmul(r_ps[:], ones8[:], c0_sb[:], start=False, stop=True)
              r_bf = consts.tile([16, U], bf16, tag="rbf")
              nc.vector.memset(r_bf[:], 0.0)
              nc.scalar.activation(r_bf[0:PB, :], r_ps[:], AF.Copy)
              rT2 = consts.tile([128, 2, 16], bf16, tag="rT2")
              nc.sync.dma_start(out=rT2[:], in_=r_bf[:], transpose=True)

              statet_sb = consts.tile([128, PB, NT], f32, tag="statet")
              nc.scalar.dma_start(out=statet_sb[:], in_=bass.AP(
                  tensor=statet_d, offset=0,
                  ap=[[PB * NT, 128], [NT, PB], [1, NT]]))

              # ---- per-batch pipeline ----
              def load_b(b):
                  st = {}
                  mt8 = mt8p.tile([128, 4, T], e4, tag="mt8", name=f"mt8_{b}")
                  nc.sync.dma_start(out=mt8[:], in_=memt8_d.ap()[b])
                  nat = natp.tile([128, NT, ENC], e3, tag="nat", name=f"nat{b}")
                  nc.sync.dma_start(out=nat[:], in_=bass.AP(
                      tensor=nat83_d, offset=b * NT * 128 * ENC,
                      ap=[[ENC, 128], [128 * ENC, NT], [1, ENC]]))
                  sh = shp.tile([K, T], bf16, tag="sh", name=f"sh{b}")
                  nc.sync.dma_start(out=sh[:], in_=bass.AP(
                      tensor=spadb_d, offset=b * (T + 2 * PAD),
                      ap=[[1, K], [1, T]]))
                  st.update(mt8=mt8, nat=nat, sh=sh)
                  return st

              def attn_b(b, st):
                  """Attention matmuls + tanh + energy; no slow round-trips on
                  the PE queue. Leaves en2d (energy rows, bf16) in st."""
                  mt8, sh = st["mt8"], st["sh"]
                  h_tiles = [[None] * NBLK for _ in range(2)]
                  for vch in range(2):
                      uo = vch * 128
                      for tb in range(NBLK):
                          to = tb * 512
                          ps = psA.tile([128, 512], f32, tag="pre")
                          nc.tensor.matmul(ps[:], wmwe8_sb[:, 0:2, uo:uo + 128],
                                           mt8[:, 0:2, to:to + 512],
                                           start=True, stop=False, perf_mode=DR)
                          nc.tensor.matmul(ps[:], wmwe8_sb[:, 2:4, uo:uo + 128],
                                           mt8[:, 2:4, to:to + 512],
                                           start=False, stop=False, perf_mode=DR)
                          nc.tensor.matmul(ps[:], cw_sb[:, uo:uo + 128],
                                           sh[:, to:to + 512],
                                           start=False, stop=True)
                          hb = hbp.tile([128, 512], bf16, tag=f"h{vch}{tb}")
                          nc.scalar.activation(hb[:], ps[:], AF.Tanh,
                                               bias=rT2[:, vch, b:b + 1],
                                               scale=1.0 / 64)
                          h_tiles[vch][tb] = hb
                  # 4 col-tiled energy matvecs run concurrently on the PE
                  en_ps = psB.tile([128, 512], f32, tag="en")
                  for vch in range(2):
                      for tb in range(NBLK):
                          nc.tensor.matmul(
                              en_ps[32 * tb:32 * tb + 1, :],
                              vaT_sb[:, vch:vch + 1], h_tiles[vch][tb][:],
                              start=(vch == 0), stop=(vch == 1),
                              tile_position=(0, 32 * tb),
                              skip_group_check=True)
                  en_rows = rowp.tile([128, 512], bf16, tag="enrows")
                  en2d = rowp.tile([NT, 128], bf16, tag="en2d",
                                   name=f"en2d{b}", bufs=3)
                  for tb in range(NBLK):
                      nc.vector.tensor_copy(en_rows[32 * tb:32 * tb + 1, :],
                                            en_ps[32 * tb:32 * tb + 1, :])
                      nc.scalar.dma_start(out=en2d[4 * tb:4 * tb + 4, :],
                                          in_=en_rows[32 * tb:32 * tb + 1, :])
                  st["en2d"] = en2d

              def wfin_b(b, st):
                  """Finish w = state + s/sum(s): emitted one batch late so the
                  PE transpose/sum never stall the PE queue."""
                  enT_ps = psC.tile([128, NT], bf16, tag="misc")
                  nc.tensor.matmul(enT_ps[:], st["en2d"][:], idb_sb[:],
                                   is_transpose=True)
                  sT = rowp.tile([128, NT], bf16, tag="sT", name=f"sT{b}",
                                 bufs=3)
                  ssumP = rowp.tile([128, 1], f32, tag="ssumP")
                  nc.scalar.activation(sT[:], enT_ps[:], AF.Sigmoid,
                                       accum_out=ssumP[:])
                  tot_ps = psC.tile([1, 1], f32, tag="misc")
                  nc.tensor.matmul(tot_ps[:], ssumP[:], ones_col[:],
                                   start=True, stop=True)
                  tot = rowp.tile([1, 1], f32, tag="tot")
                  nc.vector.tensor_copy(tot[:], tot_ps[:])
                  rec = rowp.tile([1, 1], f32, tag="rec", name=f"rec{b}",
                                  bufs=3)
                  nc.vector.reciprocal(rec[:], tot[:])
                  recB_ps = psC.tile([128, 1], f32, tag="misc")
                  nc.tensor.matmul(recB_ps[:], ones_row[:], rec[:],
                                   start=True, stop=True)
                  recB = rowp.tile([128, 1], f32, tag="recB", name=f"recB{b}",
                                   bufs=3)
                  nc.vector.tensor_copy(recB[:], recB_ps[:])
                  wT = wtp.tile([128, NT], bf16, tag="wT", name=f"wT{b}")
                  nc.vector.scalar_tensor_tensor(
                      wT[:], in0=sT[:], scalar=recB[:],
                      in1=statet_sb[:, b, :],
                      op0=ALU.mult, op1=ALU.add)
                  st["wT"] = wT

              def context_group(g, sts):
                  ctx_ps = psD.tile([128, 512], f32, tag="ctx")
                  for ch in range(NT):
                      for j in range(4):
                          stj = sts[4 * g + j]
                          nc.tensor.matmul(
                              ctx_ps[32 * j:32 * j + 1, :],
                              stj["wT"][:, ch:ch + 1],
                              stj["nat"][:, ch, :],
                              start=(ch == 0), stop=(ch == NT - 1),
                              tile_position=(0, 32 * j),
                              skip_group_check=True)
                  cv_sb = rowp.tile([128, 512], f32, tag="cvsb")
                  for j in range(4):
                      nc.vector.tensor_copy(cv_sb[32 * j:32 * j + 1, :],
                                            ctx_ps[32 * j:32 * j + 1, :])
                  for j in range(4):
                      nc.scalar.dma_start(
                          out=call_sb[4 * g + j:4 * g + j + 1, :],
                          in_=cv_sb[32 * j:32 * j + 1, :])

              sts = [load_b(0)]
              pend = None
              for b in range(PB):
                  if b + 1 < PB:
                      sts.append(load_b(b + 1))
                  attn_b(b, sts[b])
                  if pend is not None:
                      wfin_b(pend, sts[pend])
                  pend = b
                  if b == 5:
                      context_group(0, sts)
              wfin_b(7, sts[7])
              context_group(1, sts)

              # ---- final: context = Call @ Wm + sig_row^T * bm ----
              callT = []
              for ch in range(4):
                  pst = psC.tile([128, PB], f32, tag="misc")
                  nc.tensor.matmul(pst[:], call_sb[:, ch * 128:(ch + 1) * 128],
                                   idf_sb[0:PB, 0:PB], is_transpose=True)
                  t_ = consts.tile([128, PB], f32, tag=f"callT{ch}")
                  nc.vector.tensor_copy(t_[:], pst[:])
                  callT.append(t_)
              ctx_ps = psC.tile([PB, U], f32, tag="misc")
              for ch in range(4):
                  nc.tensor.matmul(ctx_ps[:], callT[ch][:], wm_sb[ch][:],
                                   start=(ch == 0), stop=False)
              nc.tensor.matmul(ctx_ps[:], sig_row[:], bm_sb[:],
                               start=False, stop=True)
              ctx_sb = consts.tile([PB, U], f32, tag="ctx")
              nc.vector.tensor_copy(ctx_sb[:], ctx_ps[:])
              nc.sync.dma_start(out=out_d.ap(), in_=ctx_sb[:])

          for _rep in range(repeat):
              _body()
    nc.compile()
    return nc


def _host_prep(inputs):
    """Fold weights on host (weight-only transforms) and shard per core."""
    f32 = np.float32
    Wq = np.asarray(inputs["Wq"], f32)
    bq = np.asarray(inputs["bq"], f32)
    Wm = np.asarray(inputs["Wm"], f32)
    bm = np.asarray(inputs["bm"], f32)
    Wl = np.asarray(inputs["Wl"], f32)
    bl = np.asarray(inputs["bl"], f32)
    conv_w = np.asarray(inputs["conv_w"], f32)
    conv_b = np.asarray(inputs["conv_b"], f32)
    We = np.asarray(inputs["We"], f32)
    be = np.asarray(inputs["be"], f32)
    v_a = np.asarray(inputs["v_a"], f32)

    WmWe = (Wm @ We).astype(f32)
    WlWe = (Wl @ We).astype(f32)
    CW64 = (conv_w[:, 0, :].T @ WlWe * 64.0).astype(f32)
    c0 = ((bq + bm + bl) @ We + be + conv_b @ WlWe).astype(f32)

    query = np.asarray(inputs["query"], f32)
    state = np.asarray(inputs["state"], f32)
    memory = np.ascontiguousarray(np.asarray(inputs["memory"], f32))

    spad = np.zeros((B, T + 2 * PAD), f32)
    spad[:, PAD:PAD + T] = state
    q1 = np.ascontiguousarray(query[:, 1, :])

    ident = np.eye(128, dtype=f32)
    shared = {
        "wmwe8": (WmWe * 32.0).astype(E4),
        "cw": CW64.astype(BF16),
        "c0": c0.reshape(1, U).astype(BF16),
        "wq": Wq.astype(BF16),
        "we": We.astype(BF16),
        "wm": Wm.astype(f32),
        "bm": bm.reshape(1, U).astype(f32),
        "vaT": np.ascontiguousarray(v_a.reshape(2, 128).T).astype(BF16),
        "idf": ident,
        "idb": np.eye(NT, dtype=np.float32).astype(BF16),
    }
    in_maps = []
    for c in range(N_CORES):
        sl = slice(c * PB, (c + 1) * PB)
        m = dict(shared)
        mb = memory[sl]
        m["memt8"] = np.ascontiguousarray(
            (mb * 2.0).reshape(PB, T, 4, 128).transpose(0, 3, 2, 1)).astype(E4)
        m["nat83"] = np.ascontiguousarray(
            mb.reshape(PB, NT, 128, ENC)).astype(E3)
        m["spad"] = np.ascontiguousarray(spad[sl])
        m["spadb"] = np.ascontiguousarray(spad[sl]).astype(BF16)
        m["statet"] = np.ascontiguousarray(
            state[sl].reshape(PB, NT, 128).transpose(2, 0, 1))
        m["q1"] = np.ascontiguousarray(q1[sl])
        in_maps.append(m)
    return in_maps


def kernel(**inputs) -> np.ndarray:
    global LAST_RESULTS
    from concourse import bass_utils

    if "nc" not in _BUILT:
        _BUILT["nc"] = _build_nc()
    nc = _BUILT["nc"]

    in_maps = _host_prep(inputs)
    res = bass_utils.run_bass_kernel_spmd(
        nc, in_maps, core_ids=list(range(N_CORES)), trace=TRACE)
    LAST_RESULTS = res
    out = np.concatenate([res.results[c]["out"] for c in range(N_CORES)], axis=0)
    return out.reshape(B, 1, U).astype(np.float32)
